# revision 85
# baseline (speedup 1.0000x reference)
"""Self-contained Trainium2 Bass kernel for nn_Attention_41472204210330.

Multi-head attention (B=2, T=2048, HIDDEN=1024, 16 heads, head_dim=64, fp32)
with RoPE, sharded over 8 NeuronCores: data-parallel over the batch (2) x
tensor-parallel over heads (4 groups of 4 heads).  Each core computes its
batch's q/k/v projections for its 4 heads, RoPE, attention, and a partial
output projection (its heads' slice of wo); the host sums the 4 partials per
batch element.

Layout (everything stays in [feature, token] "transposed" layout on chip so
the softmax reduction lands on the matmul contraction axis):
  - host pre-transposes x -> xT [1024, 2048] and the weight slices; matmul
    operands are fp16 (1 PE cycle/row, 3 more mantissa bits than bf16),
    except the attention probabilities/values which are bf16 (the scalar
    engine's exp eviction is measurably slower writing fp16).  All
    accumulation is fp32 in PSUM; end-to-end rel err ~1e-3.
  - qT/kT [256, 2048] come straight out of the projection matmuls; RoPE is
    applied during PSUM eviction (rotate-half = partition shift via
    SBUF->SBUF DMA, sin sign pre-baked on host).
  - scores are computed as ST = K Q^T [k_tok, q_tok]; exp() is applied while
    evicting PSUM on the scalar engine (scale=1/sqrt(64) folded in).  No max
    subtraction: scores stay within +-10, exp within bf16 range.
  - the softmax denominator comes from augmenting V with a ones column:
    OT_aug [65, q] = V_aug^T @ P^T, row 64 = sum_k exp.
  - normalization: ot is evicted once to SBUF (fp16) freeing its PSUM bank;
    the denominator row is bounced through DRAM into [128 x W/128] so the
    reciprocal runs on all DVE lanes, then broadcast back across the 64 dim
    partitions with a stride-0 DRAM read.  The three stages are deferred
    across attend boundaries so DVE never blocks on a DMA round trip.
  - output projection produces yT [1024, 2048] fp16 partials, summed on
    host.

Scheduling: two rooflines — the scalar engine's exp stream (16.8M elements
= ~110us at 1 col/1.2GHz cycle) and the PE matmul stream (393k rows = 164us
at the full 2.4GHz p-state).  The PE only reaches/holds 2.4GHz when
continuously busy, so the schedule is built around PE density: dummy warmup
matmuls ramp the clock while the input DMAs land, only k-tile0/q-half0
projections run before attention starts, every remaining projection (v,
second head pair, first output half) drips into attention loops as fillers
with hard in-order deadlines, scores are emitted one k-tile ahead of the
attn@v matmuls, and the score PSUM is double-buffered so exp(kt) overlaps
scores(kt+1).  The last attend is split into two 512-column halves so all
but the final eighth of the output projection overlaps attention.
"""

import sys

if "/opt/trn_rl_repo" not in sys.path:
    sys.path.insert(0, "/opt/trn_rl_repo")

import numpy as np

import bass_rust
import concourse.bass as bass
import concourse.mybir as mybir
import concourse.tile as tile

HIDDEN = 1024
NUM_HEADS = 16
D = 64  # head dim
B = 2
T = 2048
N_CORES = 8
HPC = NUM_HEADS // (N_CORES // B)  # heads per core = 4
HD = HPC * D  # per-core head dims = 256
P = 128
F32 = mybir.dt.float32
F16 = mybir.dt.float16
BF16 = mybir.dt.bfloat16

IC_CH = HIDDEN // P  # 8 input-channel chunks
NKT = T // P  # 16 k tiles
VW = D + 1  # v columns per head incl. ones column
NQ = 1024  # attend tile width (q columns per attend call)
TB = 512  # projection block width


def _split_waits(nc):
    """The in-container walrus caps semaphore waits per instruction lower
    than bass_rust/Tile assume ("Too many sync wait commands").  Hoist all
    but one semaphore wait per instruction onto nop instructions inserted
    just before it in the same engine's program order (semantically
    identical: all waits still complete before the instruction runs)."""
    from concourse._compat import not_none

    def make_nop(engine, wait):
        nop = nc.engines[engine].nop(nofuse=True)
        nop.ins.sync_info = bass_rust.SyncInfo(on_wait=[wait], on_update=[])
        return nop.ins

    tail_bb = not_none(nc.cur_bb).bb
    plans = []
    for fn in nc.m.functions:
        for bb in fn.blocks:
            plan = {}
            for inst in bb.instructions:
                si = inst.sync_info
                waits = list(si.on_wait) if si and si.on_wait else []
                sem = [w for w in waits if w.sync_type == "semaphore"]
                if len(sem) > 1:
                    plan[inst.name] = sem[:-1]
            if plan:
                plans.append((bb, plan))
    created = {}
    n_tail_before = len(tail_bb.instructions)
    for bb, plan in plans:
        eng_of = {i.name: i.engine for i in bb.instructions}
        for iname, hoists in plan.items():
            created[iname] = [make_nop(eng_of[iname], w) for w in hoists]
    created_names = {n.name for nops in created.values() for n in nops}
    tail_insts = [i for i in tail_bb.instructions if i.name not in created_names]
    assert len(tail_insts) == n_tail_before
    tail_bb.instructions = tail_insts
    for bb, plan in plans:
        out = []
        for inst in bb.instructions:
            if inst.name in plan:
                hoisted = plan[inst.name]
                out.extend(created[inst.name])
                si = inst.sync_info
                si.on_wait = [w for w in si.on_wait if w not in hoisted]
            out.append(inst)
        bb.instructions = out


def build_kernel():
    nc = bass.Bass("TRN2", target_bir_lowering=False, debug=False)

    xT = nc.dram_tensor("xT", [HIDDEN, T], F16, kind="ExternalInput")
    wq_t = nc.dram_tensor("wq_t", [HIDDEN, HD], F16, kind="ExternalInput")
    wk_t = nc.dram_tensor("wk_t", [HIDDEN, HD], F16, kind="ExternalInput")
    wv_t = nc.dram_tensor("wv_t", [HIDDEN, HD], F16, kind="ExternalInput")
    wo_t = nc.dram_tensor("wo_t", [HD, HIDDEN], F16, kind="ExternalInput")
    # column-permuted (rotate-half) q/k weight slices for head pair 0:
    # rot_half(W x) == (P W) x, so the head-critical first RoPE blocks can
    # compute the rotated projection as a second matmul instead of a
    # partition-shift copy chain (which costs ~20us of cross-engine
    # latency at the head while the PE is idle waiting on x anyway).
    wq_rot0 = nc.dram_tensor("wq_rot0", [HIDDEN, P], F16, kind="ExternalInput")
    wk_rot0 = nc.dram_tensor("wk_rot0", [HIDDEN, P], F16, kind="ExternalInput")
    cos2 = nc.dram_tensor("cos2", [P, T], F16, kind="ExternalInput")
    sin2 = nc.dram_tensor("sin2", [P, T], F16, kind="ExternalInput")
    yT = nc.dram_tensor("yT", [HIDDEN, T], F16, kind="ExternalOutput")

    mm = nc.tensor.matmul

    with tile.TileContext(nc) as tc:
        with (
            nc.allow_low_precision(
                reason="fp16/bf16 matmul operands (fp32 PSUM accumulation); "
                "rel err ~1e-3 end to end"
            ),
            tc.tile_pool(name="persist", bufs=1) as persist,
            tc.tile_pool(name="rope_pool", bufs=2) as rope_pool,
            tc.tile_pool(name="pt_pool", bufs=4) as pt_pool,
            tc.tile_pool(name="nrm_pool", bufs=2) as nrm_pool,
            tc.tile_pool(name="ysb_pool", bufs=4) as ysb_pool,
            # PSUM budget (8 banks): scores double-buffered 2x[128,1024]
            # (4 banks) + attn-out accumulator [65,1024] (2 banks) +
            # projection/output psum 2x[128,512] (2 banks).
            tc.tile_pool(name="psum_st", bufs=2, space="PSUM") as psum_st,
            tc.tile_pool(name="psum_ot", bufs=1, space="PSUM") as psum_ot,
            tc.tile_pool(name="psum_pj", bufs=2, space="PSUM") as psum_pj,
            tc.tile_pool(name="dram_pool", bufs=3, space="DRAM") as dram_pool,
        ):
            # ---- persistent SBUF tensors --------------------------------
            qTr = [
                persist.tile([P, T], F16, tag=f"qTr{m}", name=f"qTr{m}")
                for m in range(2)
            ]
            kTr = [
                persist.tile([P, T], F16, tag=f"kTr{m}", name=f"kTr{m}")
                for m in range(2)
            ]
            # per k-tile V tiles: [:, h*65:h*65+64] = v dims for head h,
            # column h*65+64 = ones (softmax denominator trick).
            v_sb = [
                persist.tile([P, HPC * VW], BF16, tag=f"v{kt}", name=f"v{kt}")
                for kt in range(NKT)
            ]
            otn = [
                persist.tile([P, T], F16, tag=f"otn{m}", name=f"otn{m}")
                for m in range(2)
            ]
            wo_sb = persist.tile([P, 2, HIDDEN], F16, tag="wo_sb", name="wo_sb")
            # per-512-col-chunk cos/sin tiles: separate tiles keep the
            # dependency granular, so the first RoPE multiply only waits
            # for chunk 0, not the last-loaded chunk.
            cos_sb = [
                persist.tile([P, TB], F16, tag=f"cos{t}", name=f"cos{t}")
                for t in range(4)
            ]
            sin_sb = [
                persist.tile([P, TB], F16, tag=f"sin{t}", name=f"sin{t}")
                for t in range(4)
            ]
            w_sbs = {}
            for name in ("q", "k", "v"):
                w_sbs[name] = persist.tile(
                    [P, IC_CH, HD], F16, tag=f"w_{name}", name=f"w_{name}"
                )
            wr_sbs = {}
            for name in ("q", "k"):
                wr_sbs[name] = persist.tile(
                    [P, IC_CH, P], F16, tag=f"wr_{name}", name=f"wr_{name}"
                )
            x_sb = [
                persist.tile([P, T], F16, tag=f"x{c}", name=f"x{c}")
                for c in range(IC_CH)
            ]

            # ---- engine warmups -----------------------------------------
            # memset whole v tiles to 1.0: gives the ones columns for the
            # denominator trick (the data regions are overwritten by the
            # v-projection evicts) and initialized operands for the PE
            # warmup below.
            for kt in range(NKT):
                nc.gpsimd.memset(
                    v_sb[kt][:].bitcast(mybir.dt.uint16), 0x3F80
                )
            # preload the scalar engine's Exp table so the first real exp
            # doesn't eat the 1.5us ACT_TABLE_LOAD.
            warm = nrm_pool.tile([P, 4], F32, tag="warm", name="warm")
            nc.vector.memset(warm[:], 0.0)
            warm2 = nrm_pool.tile([P, 4], BF16, tag="warm2", name="warm2")
            nc.scalar.activation(
                out=warm2[:], in_=warm[:],
                func=mybir.ActivationFunctionType.Exp,
            )

            # ---- input preloads -----------------------------------------
            # Only sync(SP) and scalar(Activation) are real async DMA
            # queues (gpsimd "DMAs" execute as DIRECT2D on the Pool
            # engine).  The load is aggregate-HBM-BW bound, so priority
            # order is what matters: wk + x chunks gate the first
            # projections; wq, cos/sin trail just in time for q-proj and
            # the first RoPE; wv/wo later.
            # cos/sin chunk 0 and wk first (they gate the first RoPE and
            # the first matmuls and are small).  x is loaded in COLUMN
            # quarters: the first projection blocks only read x columns
            # 0-511, so delivering those first lets the PE start ~15us
            # earlier than waiting for whole [128,2048] chunk transfers.
            def xh(c, t):
                cs = slice(t * NQ, (t + 1) * NQ)
                return dict(out=x_sb[c][:, cs], in_=xT[c * P : (c + 1) * P, cs])

            nc.sync.dma_start(out=cos_sb[0][:], in_=cos2[:, 0:TB])
            nc.sync.dma_start(out=sin_sb[0][:], in_=sin2[:, 0:TB])
            for c in range(IC_CH):
                nc.sync.dma_start(
                    out=w_sbs["k"][:, c, :], in_=wk_t[c * P : (c + 1) * P, :]
                )
            for c in range(IC_CH):
                nc.sync.dma_start(
                    out=wr_sbs["k"][:, c, :],
                    in_=wk_rot0[c * P : (c + 1) * P, :],
                )
            for c in (0, 2, 4, 6):
                nc.sync.dma_start(**xh(c, 0))
            for c in (1, 3, 5, 7):
                nc.scalar.dma_start(**xh(c, 0))
            for c in range(IC_CH):
                nc.scalar.dma_start(
                    out=w_sbs["q"][:, c, :], in_=wq_t[c * P : (c + 1) * P, :]
                )
            for c in range(IC_CH):
                nc.scalar.dma_start(
                    out=wr_sbs["q"][:, c, :],
                    in_=wq_rot0[c * P : (c + 1) * P, :],
                )
            nc.scalar.dma_start(out=cos_sb[1][:], in_=cos2[:, TB : 2 * TB])
            nc.scalar.dma_start(out=sin_sb[1][:], in_=sin2[:, TB : 2 * TB])
            # wv, the second x half, and the last cos/sin chunks are
            # deliberately issued AFTER the preamble emission below: the
            # first RoPE rotate-half DMAs must not queue behind them on
            # the rings.  wo is deferred even further into the schedule.

            # PE warmup: ~36 matmuls on v_sb garbage (gated only on the
            # memsets) keep the PE continuously busy from ~8us so its
            # p-state ramps to 2.4GHz while the x DMAs land; the real
            # projections then start at full clock.
            for i in range(48):
                dps = psum_pj.tile([P, HD], F32, tag="pj", name="dps")
                mm(
                    dps[:],
                    v_sb[i % 4][:, 0:P],
                    v_sb[4 + (i % 4)][:, 0:HD],
                    start=True,
                    stop=True,
                )

            # ---- projection generators ----------------------------------
            def project_qk_gen(name, dst, m, rot_pool_engine=False, pools=None,
                               wrot_blocks=()):
                """q/k projection for head-pair m in 512-col blocks; yields
                after every matmul (~512 PE cycles) so filler pulls never
                open a >1us hole in the exp cadence.  RoPE eviction:
                PSUM-touching ops on DVE, the SBUF-only mul/add alternates
                gpsimd/DVE per block so consecutive blocks' RoPE chains
                overlap instead of serializing on one engine.  `pools`
                optionally assigns a distinct psum pool per block (the
                preamble spreads its four blocks over four free slots so
                none waits on another's rope eviction).  Rotate-half
                shifts ride the sync DMA ring for the head-critical head
                pair 0, Pool DIRECT2D for head pair 1."""
                w_sb = w_sbs[name]
                for t in range(T // TB):
                    cs = slice(t * TB, (t + 1) * TB)
                    if t in wrot_blocks:
                        # head-critical blocks: rotated projection as a
                        # second matmul into the second bank of ONE
                        # 2-bank st-pool tile (so a block pair occupies a
                        # single slot); RoPE eviction is then three
                        # engine ops with no partition-shift copies.
                        ps2 = psum_st.tile([P, NQ], F32, tag="st", name="ps2")
                        ps, psr = ps2[:, 0:TB], ps2[:, TB:NQ]
                        wr_sb = wr_sbs[name]
                        for c in range(IC_CH):
                            mm(
                                ps,
                                w_sb[:, c, m * P : (m + 1) * P],
                                x_sb[c][:, cs],
                                start=(c == 0),
                                stop=(c == IC_CH - 1),
                            )
                            yield
                            mm(
                                psr,
                                wr_sb[:, c, :],
                                x_sb[c][:, cs],
                                start=(c == 0),
                                stop=(c == IC_CH - 1),
                            )
                            if c < IC_CH - 1:
                                yield
                        nc.vector.tensor_mul(
                            out=dst[m][:, cs], in0=ps, in1=cos_sb[t][:]
                        )
                        tmp = rope_pool.tile([P, TB], F32, tag="tmp",
                                             name="tmp")
                        nc.vector.tensor_mul(
                            out=tmp[:], in0=psr, in1=sin_sb[t][:]
                        )
                        nc.gpsimd.tensor_add(
                            out=dst[m][:, cs], in0=dst[m][:, cs], in1=tmp[:]
                        )
                        yield
                        continue
                    pool = psum_pj if pools is None else pools[t]
                    tag = "pj" if pool is psum_pj else "st"
                    pad = [P, TB] if pool is psum_pj else [P, NQ]
                    ps = pool.tile([P, TB], F32, tag=tag, name="ps",
                                   padded_shape=pad)
                    for c in range(IC_CH):
                        mm(
                            ps[:],
                            w_sb[:, c, m * P : (m + 1) * P],
                            x_sb[c][:, cs],
                            start=(c == 0),
                            stop=(c == IC_CH - 1),
                        )
                        if c < IC_CH - 1:
                            yield
                    # RoPE: out = q*cos + rotate_half(q)*sin.  The
                    # rotate-half partition shift runs as Pool DIRECT2D
                    # on fp16 tiles: SBUF->SBUF copies on the DMA rings
                    # take 5-6us each under preload traffic, the Pool
                    # engine does them in ~300ns.
                    qsb = rope_pool.tile([P, TB], F16, tag="qsb", name="qsb")
                    nc.vector.tensor_copy(out=qsb[:], in_=ps[:])
                    rot = rope_pool.tile([P, TB], F16, tag="rot", name="rot")
                    for blk in range(4):
                        src = (blk ^ 1) * 32  # swap 32-row halves
                        nc.gpsimd.dma_start(
                            out=rot[blk * 32 : blk * 32 + 32, :],
                            in_=qsb[src : src + 32, :],
                        )
                    nc.vector.tensor_mul(
                        out=dst[m][:, cs], in0=ps[:], in1=cos_sb[t][:]
                    )
                    tmp = rope_pool.tile([P, TB], F32, tag="tmp", name="tmp")
                    eng = nc.vector if t % 2 == 1 else nc.gpsimd
                    eng.tensor_mul(out=tmp[:], in0=rot[:], in1=sin_sb[t][:])
                    eng.tensor_add(
                        out=dst[m][:, cs], in0=dst[m][:, cs], in1=tmp[:]
                    )
                    yield

            def project_v_gen():
                """v projection; yields once per k-tile (~2048 PE cycles).
                Eviction is a single strided fp32->bf16 copy that scatters
                the 4 head slices around the ones columns."""
                for kt in range(NKT):
                    psv = psum_pj.tile([P, HD], F32, tag="pj", name="psv")
                    for c in range(IC_CH):
                        mm(
                            psv[:],
                            x_sb[c][:, kt * P : (kt + 1) * P],
                            w_sbs["v"][:, c, :],
                            start=(c == 0),
                            stop=(c == IC_CH - 1),
                        )
                        if c % 2 == 1 and c < IC_CH - 1:
                            yield
                    dst_ap = bass.AP(
                        tensor=v_sb[kt].tensor,
                        offset=v_sb[kt].offset,
                        ap=[list(v_sb[kt].ap[0])] + [[VW, HPC], [1, D]],
                    )
                    src_ap = bass.AP(
                        tensor=psv.tensor,
                        offset=psv.offset,
                        ap=[list(psv.ap[0])] + [[D, HPC], [1, D]],
                    )
                    nc.vector.tensor_copy(out=dst_ap, in_=src_ap)
                    yield

            def project_out_gen(n, scalar_evict_from=99):
                """output projection for q-half n; yields once per
                (mo, 512-col) unit (~1024 PE cycles).  fp16 partials halve
                the store traffic; stores alternate sync/gpsimd.  Units >=
                scalar_evict_from evict on the scalar engine (it is idle
                after the last exp, DVE is the tail bottleneck)."""
                unit = 0
                for q4 in range(NQ // TB):
                    cs = slice(n * NQ + q4 * TB, n * NQ + (q4 + 1) * TB)
                    for mo in range(HIDDEN // P):
                        ps = psum_pj.tile([P, TB], F32, tag="pj", name="psy")
                        for c in range(2):
                            mm(
                                ps[:],
                                wo_sb[:, c, mo * P : (mo + 1) * P],
                                otn[c][:, cs],
                                start=(c == 0),
                                stop=(c == 1),
                            )
                        yield
                        ysb = ysb_pool.tile([P, TB], F16, tag="ysb", name="ysb")
                        if unit >= scalar_evict_from:
                            # tail units: evict on the (now idle) scalar
                            # engine and store via Pool DIRECT2D — both
                            # clear of the DVE/sync-ring backlog.
                            nc.scalar.copy(out=ysb[:], in_=ps[:])
                            q = nc.gpsimd
                        else:
                            nc.vector.tensor_copy(out=ysb[:], in_=ps[:])
                            q = nc.sync if mo % 2 == 0 else nc.gpsimd
                        q.dma_start(
                            out=yT[mo * P : (mo + 1) * P, cs], in_=ysb[:]
                        )
                        unit += 1
                        yield

            # ---- deferred softmax normalization -------------------------
            # Stage 1 (at attend end): copy the denominator row and evict
            # ot to SBUF fp16, freeing the PSUM bank.  Stage 2 (next
            # attend start): DVE reciprocal on the [1,W] row, then a Pool
            # engine DIRECT2D broadcasts it across the 64 dim partitions
            # (partition-stride-0 SBUF read).  Stage 3 (mid next attend):
            # the normalization multiply.  Everything stays on chip - no
            # DRAM bounce latency.
            def norm_s1(ot, W):
                # den row first: its DRAM bounce is the longest dependency
                # chain.  The bounce reshapes to [128, W/128] because DVE
                # op cost scales with the FREE-dim size only — reciprocal
                # on [1, W] costs ~5.6ns/elem serially, on [128, W/128]
                # it is ~50ns total.
                den_sb = nrm_pool.tile([1, W], F32, tag="den", name="den_sb",
                                       padded_shape=[1, NQ])
                nc.vector.tensor_copy(out=den_sb[:], in_=ot[D : D + 1, :])
                dden = dram_pool.tile([1, W], F32, tag="dden", name="dden",
                                      padded_shape=[1, NQ])
                nc.sync.dma_start(out=dden[:], in_=den_sb[:])
                denp = nrm_pool.tile([P, W // P], F32, tag="denp", name="denp",
                                     padded_shape=[P, NQ // P])
                nc.sync.dma_start(
                    out=denp[:], in_=dden.rearrange("o (p f) -> (o p) f", p=P)
                )
                ot_sb = nrm_pool.tile(
                    [D, W], F16, tag="otsb", name="ot_sb", bufs=3,
                    padded_shape=[D, NQ],
                )
                nc.vector.tensor_copy(out=ot_sb[:], in_=ot[0:D, :])
                return ot_sb, denp

            def norm_s2(state, W):
                ot_sb, denp = state
                denp2 = nrm_pool.tile(
                    [P, W // P], F32, tag="denp2", name="denp2",
                    padded_shape=[P, NQ // P],
                )
                nc.vector.reciprocal(out=denp2[:], in_=denp[:])
                drec = dram_pool.tile([1, W], F32, tag="drec", name="drec",
                                      padded_shape=[1, NQ])
                nc.sync.dma_start(
                    out=drec.rearrange("o (p f) -> (o p) f", p=P), in_=denp2[:]
                )
                rb = nrm_pool.tile([D, W], F32, tag="rb", name="rb",
                                   padded_shape=[D, NQ])
                src = drec[0:1, :]
                nc.sync.dma_start(
                    out=rb[:],
                    in_=bass.AP(
                        tensor=src.tensor,
                        offset=src.offset,
                        ap=[[0, D]] + [list(a) for a in src.ap[1:]],
                    ),
                )
                return ot_sb, rb

            def norm_s3(state, h, q0, W):
                ot_sb, rb = state
                m, r0 = h // 2, (h % 2) * D
                nc.vector.tensor_mul(
                    out=otn[m][r0 : r0 + D, q0 : q0 + W],
                    in0=ot_sb[0:D, :],
                    in1=rb[:],
                )

            # ---- attention ----------------------------------------------
            pending = []  # [state, h, q0, W, stage]

            def advance_norms(to_stage2):
                for ent in pending:
                    state, h, q0, W, stage = ent
                    if to_stage2 and stage == 1:
                        ent[0], ent[4] = norm_s2(state, W), 2
                    elif not to_stage2 and stage == 2:
                        norm_s3(state, h, q0, W)
                        ent[0], ent[4] = None, 3
                pending[:] = [e for e in pending if e[4] < 3]

            # fp16 ones row for the fast-normalization PE broadcast
            ones16 = persist.tile([1, D], F16, tag="ones16", name="ones16")
            nc.gpsimd.memset(ones16[:].bitcast(mybir.dt.uint16), 0x3C00)

            def attend(h, q0, W, pulls=None, fast_norm=False,
                       pulls_first=False):
                m, r0 = h // 2, (h % 2) * D
                advance_norms(to_stage2=True)
                ot = psum_ot.tile([VW, W], F32, tag="ot", name="ot",
                                  padded_shape=[VW, NQ])
                prev_pt = None

                def pv(pkt, ppt, stop):
                    for sub in range(W // TB):
                        ss = slice(sub * TB, (sub + 1) * TB)
                        mm(
                            ot[:, ss],
                            v_sb[pkt][:, h * VW : (h + 1) * VW],
                            ppt[:, ss],
                            start=(pkt == 0),
                            stop=stop,
                        )

                for kt in range(NKT):
                    if kt == 6:
                        advance_norms(to_stage2=False)
                    if pulls_first and pulls is not None:
                        pulls(kt)
                    st = psum_st.tile([P, W], F32, tag="st", name="st",
                                      padded_shape=[P, NQ])
                    for sub in range(W // TB):
                        c0 = q0 + sub * TB
                        mm(
                            st[:, sub * TB : (sub + 1) * TB],
                            kTr[m][r0 : r0 + D, kt * P : (kt + 1) * P],
                            qTr[m][r0 : r0 + D, c0 : c0 + TB],
                            start=True,
                            stop=True,
                        )
                    pt = pt_pool.tile([P, W], BF16, tag="pt", name="pt",
                                      padded_shape=[P, NQ])
                    # exp((K Q^T)/sqrt(64)) while evicting PSUM
                    nc.scalar.activation(
                        out=pt[:],
                        in_=st[:],
                        func=mybir.ActivationFunctionType.Exp,
                        scale=float(1.0 / np.sqrt(D)),
                    )
                    # fillers AFTER scores/exp: the exp stream leads, the
                    # attn@v for the previous k-tile and any pulled
                    # projection work keep the PE busy under it.  (att0
                    # instead pulls BEFORE scores: its first scores wait
                    # on the RoPE chain, and the 4-deep engine wait queues
                    # would otherwise park the whole stream behind them.)
                    if not pulls_first and pulls is not None:
                        pulls(kt)
                    if prev_pt is not None:
                        pv(*prev_pt, stop=False)
                    prev_pt = (kt, pt)
                pv(*prev_pt, stop=True)
                if not fast_norm:
                    pending.append([norm_s1(ot, W), h, q0, W, 1])
                    return
                # fast close for the final attend: serial reciprocal on
                # DVE + a K=1 ones-matmul broadcast on the PE replace the
                # 4-hop DRAM bounce (dummy matmuls emitted by the caller
                # keep the PE p-state up during the ~4.5us DVE chain).
                den_sb = nrm_pool.tile([1, W], F32, tag="den", name="den_sb",
                                       padded_shape=[1, NQ])
                nc.vector.tensor_copy(out=den_sb[:], in_=ot[D : D + 1, :])
                rec16 = nrm_pool.tile([1, W], F16, tag="rec16", name="rec16",
                                      padded_shape=[1, NQ])
                nc.vector.reciprocal(out=rec16[:], in_=den_sb[:])
                ot_sb = nrm_pool.tile(
                    [D, W], F16, tag="otsb", name="ot_sb", bufs=3,
                    padded_shape=[D, NQ],
                )
                nc.vector.tensor_copy(out=ot_sb[:], in_=ot[0:D, :])
                rb_ps = psum_pj.tile([D, W], F32, tag="pj", name="rb_ps")
                mm(rb_ps[:], ones16[:], rec16[:], start=True, stop=True)
                nc.vector.tensor_mul(
                    out=otn[m][r0 : r0 + D, q0 : q0 + W],
                    in0=ot_sb[:],
                    in1=rb_ps[:],
                )

            # ---- emission schedule --------------------------------------
            import itertools

            gk0 = project_qk_gen("k", kTr, 0, wrot_blocks=(0, 1))
            gq0 = project_qk_gen("q", qTr, 0, wrot_blocks=(0, 1))
            gv = project_v_gen()
            gk1 = project_qk_gen("k", kTr, 1, rot_pool_engine=True)
            gq1 = project_qk_gen("q", qTr, 1, rot_pool_engine=True)
            go0 = project_out_gen(0)
            go1 = project_out_gen(1, scalar_evict_from=8)

            def take(g, n):
                for _ in range(n):
                    next(g, None)

            # preamble: only what attend(0,0)'s first scores need — k
            # block 0 and q cols 0-1023 of the first head pair.  v and the
            # remaining k/q blocks drip into attend(0,0) with in-order
            # deadlines (scores(kt) needs k block kt//4 roped, attn@v(kt)
            # needs v tile kt evicted).
            # k blocks 0-1 and q blocks 0-1 only read x columns 0-1023
            # (the first x half); everything else drips into attend(0,0).
            # Interleaved k/q emission so the two st-pool slots hold one
            # k pair and one q pair concurrently.
            take(gk0, 16)
            take(gq0, 16)
            take(gk0, 16)
            take(gq0, 16)
            # wv lands before attend(0,0)'s first v matmuls; its
            # descriptors queue behind the preamble rots.
            for c in range(IC_CH):
                nc.scalar.dma_start(
                    out=w_sbs["v"][:, c, :], in_=wv_t[c * P : (c + 1) * P, :]
                )
            chain_a = itertools.chain(gk0, gq0)

            def pulls_00(kt):
                # the second x half and the tail cos/sin chunks are
                # emitted HERE so their ring entries queue behind the
                # preamble rots (the engine wait-queues let ready work
                # cut ahead of parked entries, so anything emitted before
                # attend(0,0) would overtake the waiting rots).
                if kt == 1:
                    for c in (0, 2, 4, 6):
                        nc.sync.dma_start(**xh(c, 1))
                    for c in (1, 3, 5, 7):
                        nc.scalar.dma_start(**xh(c, 1))
                if kt == 2:
                    for t in (2, 3):
                        cs = slice(t * TB, (t + 1) * TB)
                        nc.scalar.dma_start(out=cos_sb[t][:], in_=cos2[:, cs])
                        nc.scalar.dma_start(out=sin_sb[t][:], in_=sin2[:, cs])
                take(gv, 8 if kt < 2 else 4)
                if 2 <= kt < 10:
                    take(chain_a, 4)

            def drip(gen, kts, n=1):
                def pulls(kt):
                    if kt in kts:
                        take(gen, n)

                return pulls

            # Filler placement keeps the PE dense in EVERY attend (an idle
            # PE falls out of its 2.4GHz p-state, and at 1.2GHz the
            # attend's own matmuls no longer fit under the exp stream).
            # (3,0) runs before (2,1) so the first output half (ready
            # after (3,0)'s norm) fills the last attends; the final attend
            # is split into two 512-col halves so even most of the second
            # output half overlaps attention.
            allkt = tuple(range(NKT))
            attend(0, 0, NQ, pulls_00, pulls_first=True)
            attend(0, NQ, NQ, drip(gk1, allkt))
            nc.sync.dma_start(
                out=wo_sb[:], in_=wo_t.rearrange("(c p) o -> p c o", p=P)
            )
            attend(1, 0, NQ, drip(gk1, allkt))
            attend(1, NQ, NQ, drip(gq1, allkt))
            attend(2, 0, NQ, drip(gq1, allkt))
            attend(3, 0, NQ)
            attend(2, NQ, NQ, drip(go0, (8, 9, 10, 11, 12, 13, 14, 15), n=2))
            attend(3, NQ, TB, drip(go0, allkt))
            take(go0, 32)  # drain any remainder
            attend(3, NQ + TB, TB,
                   drip(go1, (6, 7, 8, 9, 10, 11, 12, 13), n=2),
                   fast_norm=True)
            # tail: dummy matmuls keep the PE p-state up while the fast
            # normalization's DVE chain completes, then the last quarter
            # of the output projection (scalar-engine evicts).
            for i in range(10):
                dps = psum_st.tile([P, TB], F32, tag="st", name="dps",
                                   padded_shape=[P, NQ])
                mm(
                    dps[:],
                    kTr[i % 2][:, 0:P],
                    qTr[i % 2][:, 0:TB],
                    start=True,
                    stop=True,
                )
            take(go1, 32)
    _split_waits(nc)
    return nc


def _rope_tables():
    inv_freq = 1.0 / (10000.0 ** (np.arange(0, D, 2, dtype=np.float32) / D))
    t = np.arange(T, dtype=np.float32)
    freqs = t[:, None] * inv_freq[None, :]  # [T, 32]
    emb = np.concatenate((freqs, freqs), axis=-1)  # [T, 64]
    cos = np.cos(emb).T.astype(np.float32)  # [64, T]
    sin = np.sin(emb).T.astype(np.float32)
    sign = np.where(np.arange(D) < D // 2, -1.0, 1.0).astype(np.float32)
    sin_signed = sin * sign[:, None]
    cos2 = np.ascontiguousarray(np.concatenate([cos, cos], axis=0))  # [128,T]
    sin2 = np.ascontiguousarray(np.concatenate([sin_signed, sin_signed], 0))
    return cos2, sin2


def make_in_maps(x, wq, wk, wv, wo):
    f16 = np.float16
    cos2, sin2 = _rope_tables()
    # rotate-half column permutation of the head-pair-0 weight block:
    # column j=(h*64+d) <- column h*64+((d+32)%64).  Sign comes from the
    # pre-signed sin table, matching the copy-based rotate path.
    j = np.arange(P)
    rot_src = (j // D) * D + (j % D + D // 2) % D
    in_maps = []
    for core in range(N_CORES):
        b, g = divmod(core, N_CORES // B)
        hs = slice(g * HD, (g + 1) * HD)
        wq_t = np.ascontiguousarray(wq[hs].T).astype(f16)
        wk_t = np.ascontiguousarray(wk[hs].T).astype(f16)
        in_maps.append(
            {
                "xT": np.ascontiguousarray(x[b].T).astype(f16),
                "wq_t": wq_t,
                "wk_t": wk_t,
                "wv_t": np.ascontiguousarray(wv[hs].T).astype(f16),
                "wo_t": np.ascontiguousarray(wo[:, hs].T).astype(f16),
                "wq_rot0": np.ascontiguousarray(wq_t[:, rot_src]),
                "wk_rot0": np.ascontiguousarray(wk_t[:, rot_src]),
                "cos2": cos2.astype(f16),
                "sin2": sin2.astype(f16),
            }
        )
    return in_maps


def gather_output(results):
    y = np.zeros((B, T, HIDDEN), dtype=np.float32)
    for core, res in enumerate(results):
        b = core // (N_CORES // B)
        y[b] += res["yT"].T.astype(np.float32)
    return y


_NC = None


def kernel(x, wq, wk, wv, wo):
    global _NC
    import time

    from concourse.bass_utils import run_bass_kernel_spmd

    if _NC is None:
        _NC = build_kernel()
    in_maps = make_in_maps(
        np.asarray(x), np.asarray(wq), np.asarray(wk), np.asarray(wv), np.asarray(wo)
    )
    try:
        res = run_bass_kernel_spmd(_NC, in_maps, core_ids=list(range(N_CORES)))
    except Exception:
        # transient device wedge (e.g. NRT_EXEC_UNIT_UNRECOVERABLE from a
        # prior run) -- retry once
        time.sleep(2.0)
        res = run_bass_kernel_spmd(_NC, in_maps, core_ids=list(range(N_CORES)))
    return gather_output(res.results)


# revision 91
# speedup vs baseline: 1.1906x; 1.1906x over previous
"""Self-contained Trainium2 Bass kernel for nn_Attention_41472204210330.

Multi-head attention (B=2, T=2048, HIDDEN=1024, 16 heads, head_dim=64, fp32)
with RoPE, sharded over 8 NeuronCores: data-parallel over the batch (2) x
tensor-parallel over heads (4 groups of 4 heads).  Each core computes its
batch's q/k/v projections for its 4 heads, RoPE, attention, and a partial
output projection (its heads' slice of wo); the host sums the 4 partials per
batch element.

Layout (everything stays in [feature, token] "transposed" layout on chip so
the softmax reduction lands on the matmul contraction axis):
  - host pre-transposes x -> xT [1024, 2048] and the weight slices; matmul
    operands are fp16 (1 PE cycle/row, 3 more mantissa bits than bf16),
    except the attention probabilities/values which are bf16 (the scalar
    engine's exp eviction is measurably slower writing fp16).  All
    accumulation is fp32 in PSUM; end-to-end rel err ~1e-3.
  - qT/kT [256, 2048] come straight out of the projection matmuls; RoPE is
    applied during PSUM eviction (rotate-half = partition shift via
    SBUF->SBUF DMA, sin sign pre-baked on host).
  - scores are computed as ST = K Q^T [k_tok, q_tok]; exp() is applied while
    evicting PSUM on the scalar engine (scale=1/sqrt(64) folded in).  No max
    subtraction: scores stay within +-10, exp within bf16 range.
  - the softmax denominator comes from augmenting V with a ones column:
    OT_aug [65, q] = V_aug^T @ P^T, row 64 = sum_k exp.
  - normalization: ot is evicted once to SBUF (fp16) freeing its PSUM bank;
    the denominator row is bounced through DRAM into [128 x W/128] so the
    reciprocal runs on all DVE lanes, then broadcast back across the 64 dim
    partitions with a stride-0 DRAM read.  The three stages are deferred
    across attend boundaries so DVE never blocks on a DMA round trip.
  - output projection produces yT [1024, 2048] fp16 partials, summed on
    host.

Scheduling: two rooflines — the scalar engine's exp stream (16.8M elements
= ~110us at 1 col/1.2GHz cycle) and the PE matmul stream (393k rows = 164us
at the full 2.4GHz p-state).  The PE only reaches/holds 2.4GHz when
continuously busy, so the schedule is built around PE density: dummy warmup
matmuls ramp the clock while the input DMAs land, only k-tile0/q-half0
projections run before attention starts, every remaining projection (v,
second head pair, first output half) drips into attention loops as fillers
with hard in-order deadlines, scores are emitted one k-tile ahead of the
attn@v matmuls, and the score PSUM is double-buffered so exp(kt) overlaps
scores(kt+1).  The last attend is split into two 512-column halves so all
but the final eighth of the output projection overlaps attention.
"""

import sys

if "/opt/trn_rl_repo" not in sys.path:
    sys.path.insert(0, "/opt/trn_rl_repo")

import numpy as np

import bass_rust
import concourse.bass as bass
import concourse.mybir as mybir
import concourse.tile as tile

HIDDEN = 1024
NUM_HEADS = 16
D = 64  # head dim
B = 2
T = 2048
N_CORES = 8
HPC = NUM_HEADS // (N_CORES // B)  # heads per core = 4
HD = HPC * D  # per-core head dims = 256
P = 128
F32 = mybir.dt.float32
F16 = mybir.dt.float16
BF16 = mybir.dt.bfloat16

IC_CH = HIDDEN // P  # 8 input-channel chunks
NKT = T // P  # 16 k tiles
VW = D + 1  # v columns per head incl. ones column
NQ = 1024  # attend tile width (q columns per attend call)
TB = 512  # projection block width


def _split_waits(nc):
    """The in-container walrus caps semaphore waits per instruction lower
    than bass_rust/Tile assume ("Too many sync wait commands").  Hoist all
    but one semaphore wait per instruction onto nop instructions inserted
    just before it in the same engine's program order (semantically
    identical: all waits still complete before the instruction runs)."""
    from concourse._compat import not_none

    def make_nop(engine, wait):
        nop = nc.engines[engine].nop(nofuse=True)
        nop.ins.sync_info = bass_rust.SyncInfo(on_wait=[wait], on_update=[])
        return nop.ins

    tail_bb = not_none(nc.cur_bb).bb
    plans = []
    for fn in nc.m.functions:
        for bb in fn.blocks:
            plan = {}
            for inst in bb.instructions:
                si = inst.sync_info
                waits = list(si.on_wait) if si and si.on_wait else []
                sem = [w for w in waits if w.sync_type == "semaphore"]
                if len(sem) > 1:
                    plan[inst.name] = sem[:-1]
            if plan:
                plans.append((bb, plan))
    created = {}
    n_tail_before = len(tail_bb.instructions)
    for bb, plan in plans:
        eng_of = {i.name: i.engine for i in bb.instructions}
        for iname, hoists in plan.items():
            created[iname] = [make_nop(eng_of[iname], w) for w in hoists]
    created_names = {n.name for nops in created.values() for n in nops}
    tail_insts = [i for i in tail_bb.instructions if i.name not in created_names]
    assert len(tail_insts) == n_tail_before
    tail_bb.instructions = tail_insts
    for bb, plan in plans:
        out = []
        for inst in bb.instructions:
            if inst.name in plan:
                hoisted = plan[inst.name]
                out.extend(created[inst.name])
                si = inst.sync_info
                si.on_wait = [w for w in si.on_wait if w not in hoisted]
            out.append(inst)
        bb.instructions = out


def build_kernel():
    nc = bass.Bass("TRN2", target_bir_lowering=False, debug=False)

    xT = nc.dram_tensor("xT", [HIDDEN, T], F16, kind="ExternalInput")
    wq_t = nc.dram_tensor("wq_t", [HIDDEN, HD], F16, kind="ExternalInput")
    wk_t = nc.dram_tensor("wk_t", [HIDDEN, HD], F16, kind="ExternalInput")
    wv_t = nc.dram_tensor("wv_t", [HIDDEN, HD], F16, kind="ExternalInput")
    wo_t = nc.dram_tensor("wo_t", [HD, HIDDEN], F16, kind="ExternalInput")
    cos2 = nc.dram_tensor("cos2", [P, T], F16, kind="ExternalInput")
    sin2 = nc.dram_tensor("sin2", [P, T], F16, kind="ExternalInput")
    yT = nc.dram_tensor("yT", [HIDDEN, T], F16, kind="ExternalOutput")

    mm = nc.tensor.matmul

    with tile.TileContext(nc) as tc:
        with (
            nc.allow_low_precision(
                reason="fp16/bf16 matmul operands (fp32 PSUM accumulation); "
                "rel err ~1e-3 end to end"
            ),
            tc.tile_pool(name="persist", bufs=1) as persist,
            tc.tile_pool(name="rope_pool", bufs=2) as rope_pool,
            tc.tile_pool(name="pt_pool", bufs=4) as pt_pool,
            tc.tile_pool(name="nrm_pool", bufs=2) as nrm_pool,
            tc.tile_pool(name="ysb_pool", bufs=4) as ysb_pool,
            # PSUM budget (8 banks): scores double-buffered 2x[128,1024]
            # (4 banks) + attn-out accumulator [65,1024] (2 banks) +
            # projection/output psum 2x[128,512] (2 banks).
            tc.tile_pool(name="psum_st", bufs=2, space="PSUM") as psum_st,
            tc.tile_pool(name="psum_ot", bufs=1, space="PSUM") as psum_ot,
            tc.tile_pool(name="psum_pj", bufs=2, space="PSUM") as psum_pj,
            tc.tile_pool(name="dram_pool", bufs=3, space="DRAM") as dram_pool,
        ):
            # ---- persistent SBUF tensors --------------------------------
            qTr = [
                persist.tile([P, T], F16, tag=f"qTr{m}", name=f"qTr{m}")
                for m in range(2)
            ]
            kTr = [
                persist.tile([P, T], F16, tag=f"kTr{m}", name=f"kTr{m}")
                for m in range(2)
            ]
            # per k-tile V tiles: [:, h*65:h*65+64] = v dims for head h,
            # column h*65+64 = ones (softmax denominator trick).
            v_sb = [
                persist.tile([P, HPC * VW], BF16, tag=f"v{kt}", name=f"v{kt}")
                for kt in range(NKT)
            ]
            otn = [
                persist.tile([P, T], F16, tag=f"otn{m}", name=f"otn{m}")
                for m in range(2)
            ]
            wo_sb = persist.tile([P, 2, HIDDEN], F16, tag="wo_sb", name="wo_sb")
            # per-512-col-chunk cos/sin tiles: separate tiles keep the
            # dependency granular, so the first RoPE multiply only waits
            # for chunk 0, not the last-loaded chunk.
            cos_sb = [
                persist.tile([P, TB], F16, tag=f"cos{t}", name=f"cos{t}")
                for t in range(4)
            ]
            sin_sb = [
                persist.tile([P, TB], F16, tag=f"sin{t}", name=f"sin{t}")
                for t in range(4)
            ]
            w_sbs = {}
            for name in ("q", "k", "v"):
                w_sbs[name] = persist.tile(
                    [P, IC_CH, HD], F16, tag=f"w_{name}", name=f"w_{name}"
                )
            x_sb = [
                persist.tile([P, T], F16, tag=f"x{c}", name=f"x{c}")
                for c in range(IC_CH)
            ]

            # ---- engine warmups -----------------------------------------
            # memset whole v tiles to 1.0: gives the ones columns for the
            # denominator trick (the data regions are overwritten by the
            # v-projection evicts) and initialized operands for the PE
            # warmup below.
            for kt in range(NKT):
                nc.gpsimd.memset(
                    v_sb[kt][:].bitcast(mybir.dt.uint16), 0x3F80
                )
            # preload the scalar engine's Exp table so the first real exp
            # doesn't eat the 1.5us ACT_TABLE_LOAD.
            warm = nrm_pool.tile([P, 4], F32, tag="warm", name="warm")
            nc.vector.memset(warm[:], 0.0)
            warm2 = nrm_pool.tile([P, 4], BF16, tag="warm2", name="warm2")
            nc.scalar.activation(
                out=warm2[:], in_=warm[:],
                func=mybir.ActivationFunctionType.Exp,
            )

            # ---- input preloads -----------------------------------------
            # Only sync(SP) and scalar(Activation) are real async DMA
            # queues (gpsimd "DMAs" execute as DIRECT2D on the Pool
            # engine).  The load is aggregate-HBM-BW bound, so priority
            # order is what matters: wk + x chunks gate the first
            # projections; wq, cos/sin trail just in time for q-proj and
            # the first RoPE; wv/wo later.
            # cos/sin chunk 0 and wk first (they gate the first RoPE and
            # the first matmuls and are small).  x is loaded in COLUMN
            # quarters: the first projection blocks only read x columns
            # 0-511, so delivering those first lets the PE start ~15us
            # earlier than waiting for whole [128,2048] chunk transfers.
            def xh(c, t):
                cs = slice(t * NQ, (t + 1) * NQ)
                return dict(out=x_sb[c][:, cs], in_=xT[c * P : (c + 1) * P, cs])

            nc.sync.dma_start(out=cos_sb[0][:], in_=cos2[:, 0:TB])
            nc.sync.dma_start(out=sin_sb[0][:], in_=sin2[:, 0:TB])
            for c in range(IC_CH):
                nc.sync.dma_start(
                    out=w_sbs["k"][:, c, :], in_=wk_t[c * P : (c + 1) * P, :]
                )
            for c in (0, 2, 4, 6):
                nc.sync.dma_start(**xh(c, 0))
            for c in (1, 3, 5, 7):
                nc.scalar.dma_start(**xh(c, 0))
            for c in range(IC_CH):
                nc.scalar.dma_start(
                    out=w_sbs["q"][:, c, :], in_=wq_t[c * P : (c + 1) * P, :]
                )
            nc.scalar.dma_start(out=cos_sb[1][:], in_=cos2[:, TB : 2 * TB])
            nc.scalar.dma_start(out=sin_sb[1][:], in_=sin2[:, TB : 2 * TB])
            # wv, the second x half, and the last cos/sin chunks are
            # deliberately issued AFTER the preamble emission below: the
            # first RoPE rotate-half DMAs must not queue behind them on
            # the rings.  wo is deferred even further into the schedule.

            # PE warmup: ~36 matmuls on v_sb garbage (gated only on the
            # memsets) keep the PE continuously busy from ~8us so its
            # p-state ramps to 2.4GHz while the x DMAs land; the real
            # projections then start at full clock.
            for i in range(48):
                dps = psum_pj.tile([P, HD], F32, tag="pj", name="dps")
                mm(
                    dps[:],
                    v_sb[i % 4][:, 0:P],
                    v_sb[4 + (i % 4)][:, 0:HD],
                    start=True,
                    stop=True,
                )

            # ---- projection generators ----------------------------------
            def project_qk_gen(name, dst, m, rot_pool_engine=False, pools=None):
                """q/k projection for head-pair m in 512-col blocks; yields
                after every matmul (~512 PE cycles) so filler pulls never
                open a >1us hole in the exp cadence.  RoPE eviction:
                PSUM-touching ops on DVE, the SBUF-only mul/add alternates
                gpsimd/DVE per block so consecutive blocks' RoPE chains
                overlap instead of serializing on one engine.  `pools`
                optionally assigns a distinct psum pool per block (the
                preamble spreads its four blocks over four free slots so
                none waits on another's rope eviction).  Rotate-half
                shifts ride the sync DMA ring for the head-critical head
                pair 0, Pool DIRECT2D for head pair 1."""
                w_sb = w_sbs[name]
                for t in range(T // TB):
                    cs = slice(t * TB, (t + 1) * TB)
                    pool = psum_pj if pools is None else pools[t]
                    tag = "pj" if pool is psum_pj else "st"
                    pad = [P, TB] if pool is psum_pj else [P, NQ]
                    ps = pool.tile([P, TB], F32, tag=tag, name="ps",
                                   padded_shape=pad)
                    for c in range(IC_CH):
                        mm(
                            ps[:],
                            w_sb[:, c, m * P : (m + 1) * P],
                            x_sb[c][:, cs],
                            start=(c == 0),
                            stop=(c == IC_CH - 1),
                        )
                        if c < IC_CH - 1:
                            yield
                    # RoPE: out = q*cos + rotate_half(q)*sin
                    qsb = rope_pool.tile([P, TB], F32, tag="qsb", name="qsb")
                    nc.vector.tensor_copy(out=qsb[:], in_=ps[:])
                    rot = rope_pool.tile([P, TB], F32, tag="rot", name="rot")
                    rot_q = nc.gpsimd if rot_pool_engine else nc.sync
                    for blk in range(4):
                        src = (blk ^ 1) * 32  # swap 32-row halves
                        rot_q.dma_start(
                            out=rot[blk * 32 : blk * 32 + 32, :],
                            in_=qsb[src : src + 32, :],
                        )
                    nc.vector.tensor_mul(
                        out=dst[m][:, cs], in0=ps[:], in1=cos_sb[t][:]
                    )
                    tmp = rope_pool.tile([P, TB], F32, tag="tmp", name="tmp")
                    eng = nc.vector if t % 2 == 1 else nc.gpsimd
                    eng.tensor_mul(out=tmp[:], in0=rot[:], in1=sin_sb[t][:])
                    eng.tensor_add(
                        out=dst[m][:, cs], in0=dst[m][:, cs], in1=tmp[:]
                    )
                    yield

            def project_v_gen():
                """v projection; yields once per k-tile (~2048 PE cycles).
                Eviction is a single strided fp32->bf16 copy that scatters
                the 4 head slices around the ones columns."""
                for kt in range(NKT):
                    psv = psum_pj.tile([P, HD], F32, tag="pj", name="psv")
                    for c in range(IC_CH):
                        mm(
                            psv[:],
                            x_sb[c][:, kt * P : (kt + 1) * P],
                            w_sbs["v"][:, c, :],
                            start=(c == 0),
                            stop=(c == IC_CH - 1),
                        )
                        if c % 2 == 1 and c < IC_CH - 1:
                            yield
                    dst_ap = bass.AP(
                        tensor=v_sb[kt].tensor,
                        offset=v_sb[kt].offset,
                        ap=[list(v_sb[kt].ap[0])] + [[VW, HPC], [1, D]],
                    )
                    src_ap = bass.AP(
                        tensor=psv.tensor,
                        offset=psv.offset,
                        ap=[list(psv.ap[0])] + [[D, HPC], [1, D]],
                    )
                    nc.vector.tensor_copy(out=dst_ap, in_=src_ap)
                    yield

            def project_out_gen(n, scalar_evict_from=99):
                """output projection for q-half n; yields once per
                (mo, 512-col) unit (~1024 PE cycles).  fp16 partials halve
                the store traffic; stores alternate sync/gpsimd.  Units >=
                scalar_evict_from evict on the scalar engine (it is idle
                after the last exp, DVE is the tail bottleneck)."""
                unit = 0
                for q4 in range(NQ // TB):
                    cs = slice(n * NQ + q4 * TB, n * NQ + (q4 + 1) * TB)
                    for mo in range(HIDDEN // P):
                        ps = psum_pj.tile([P, TB], F32, tag="pj", name="psy")
                        for c in range(2):
                            mm(
                                ps[:],
                                wo_sb[:, c, mo * P : (mo + 1) * P],
                                otn[c][:, cs],
                                start=(c == 0),
                                stop=(c == 1),
                            )
                        yield
                        ysb = ysb_pool.tile([P, TB], F16, tag="ysb", name="ysb")
                        if unit >= scalar_evict_from:
                            # tail units: evict on the (now idle) scalar
                            # engine and store via Pool DIRECT2D — both
                            # clear of the DVE/sync-ring backlog.
                            nc.scalar.copy(out=ysb[:], in_=ps[:])
                            q = nc.gpsimd
                        else:
                            nc.vector.tensor_copy(out=ysb[:], in_=ps[:])
                            q = nc.sync if mo % 2 == 0 else nc.gpsimd
                        q.dma_start(
                            out=yT[mo * P : (mo + 1) * P, cs], in_=ysb[:]
                        )
                        unit += 1
                        yield

            # ---- deferred softmax normalization -------------------------
            # Stage 1 (at attend end): copy the denominator row and evict
            # ot to SBUF fp16, freeing the PSUM bank.  Stage 2 (next
            # attend start): DVE reciprocal on the [1,W] row, then a Pool
            # engine DIRECT2D broadcasts it across the 64 dim partitions
            # (partition-stride-0 SBUF read).  Stage 3 (mid next attend):
            # the normalization multiply.  Everything stays on chip - no
            # DRAM bounce latency.
            def norm_s1(ot, W):
                # den row first: its DRAM bounce is the longest dependency
                # chain.  The bounce reshapes to [128, W/128] because DVE
                # op cost scales with the FREE-dim size only — reciprocal
                # on [1, W] costs ~5.6ns/elem serially, on [128, W/128]
                # it is ~50ns total.
                den_sb = nrm_pool.tile([1, W], F32, tag="den", name="den_sb",
                                       padded_shape=[1, NQ])
                nc.vector.tensor_copy(out=den_sb[:], in_=ot[D : D + 1, :])
                dden = dram_pool.tile([1, W], F32, tag="dden", name="dden",
                                      padded_shape=[1, NQ])
                nc.sync.dma_start(out=dden[:], in_=den_sb[:])
                denp = nrm_pool.tile([P, W // P], F32, tag="denp", name="denp",
                                     padded_shape=[P, NQ // P])
                nc.sync.dma_start(
                    out=denp[:], in_=dden.rearrange("o (p f) -> (o p) f", p=P)
                )
                ot_sb = nrm_pool.tile(
                    [D, W], F16, tag="otsb", name="ot_sb", bufs=3,
                    padded_shape=[D, NQ],
                )
                nc.vector.tensor_copy(out=ot_sb[:], in_=ot[0:D, :])
                return ot_sb, denp

            def norm_s2(state, W):
                ot_sb, denp = state
                denp2 = nrm_pool.tile(
                    [P, W // P], F32, tag="denp2", name="denp2",
                    padded_shape=[P, NQ // P],
                )
                nc.vector.reciprocal(out=denp2[:], in_=denp[:])
                drec = dram_pool.tile([1, W], F32, tag="drec", name="drec",
                                      padded_shape=[1, NQ])
                nc.sync.dma_start(
                    out=drec.rearrange("o (p f) -> (o p) f", p=P), in_=denp2[:]
                )
                rb = nrm_pool.tile([D, W], F32, tag="rb", name="rb",
                                   padded_shape=[D, NQ])
                src = drec[0:1, :]
                nc.sync.dma_start(
                    out=rb[:],
                    in_=bass.AP(
                        tensor=src.tensor,
                        offset=src.offset,
                        ap=[[0, D]] + [list(a) for a in src.ap[1:]],
                    ),
                )
                return ot_sb, rb

            def norm_s3(state, h, q0, W):
                ot_sb, rb = state
                m, r0 = h // 2, (h % 2) * D
                nc.vector.tensor_mul(
                    out=otn[m][r0 : r0 + D, q0 : q0 + W],
                    in0=ot_sb[0:D, :],
                    in1=rb[:],
                )

            # ---- attention ----------------------------------------------
            pending = []  # [state, h, q0, W, stage]

            def advance_norms(to_stage2):
                for ent in pending:
                    state, h, q0, W, stage = ent
                    if to_stage2 and stage == 1:
                        ent[0], ent[4] = norm_s2(state, W), 2
                    elif not to_stage2 and stage == 2:
                        norm_s3(state, h, q0, W)
                        ent[0], ent[4] = None, 3
                pending[:] = [e for e in pending if e[4] < 3]

            # fp16 ones row for the fast-normalization PE broadcast
            ones16 = persist.tile([1, D], F16, tag="ones16", name="ones16")
            nc.gpsimd.memset(ones16[:].bitcast(mybir.dt.uint16), 0x3C00)

            def attend(h, q0, W, pulls=None, fast_norm=False,
                       pulls_first=False):
                m, r0 = h // 2, (h % 2) * D
                advance_norms(to_stage2=True)
                ot = psum_ot.tile([VW, W], F32, tag="ot", name="ot",
                                  padded_shape=[VW, NQ])
                prev_pt = None

                def pv(pkt, ppt, stop):
                    for sub in range(W // TB):
                        ss = slice(sub * TB, (sub + 1) * TB)
                        mm(
                            ot[:, ss],
                            v_sb[pkt][:, h * VW : (h + 1) * VW],
                            ppt[:, ss],
                            start=(pkt == 0),
                            stop=stop,
                        )

                for kt in range(NKT):
                    if kt == 6:
                        advance_norms(to_stage2=False)
                    if pulls_first and pulls is not None:
                        pulls(kt)
                    st = psum_st.tile([P, W], F32, tag="st", name="st",
                                      padded_shape=[P, NQ])
                    for sub in range(W // TB):
                        c0 = q0 + sub * TB
                        mm(
                            st[:, sub * TB : (sub + 1) * TB],
                            kTr[m][r0 : r0 + D, kt * P : (kt + 1) * P],
                            qTr[m][r0 : r0 + D, c0 : c0 + TB],
                            start=True,
                            stop=True,
                        )
                    pt = pt_pool.tile([P, W], BF16, tag="pt", name="pt",
                                      padded_shape=[P, NQ])
                    # exp((K Q^T)/sqrt(64)) while evicting PSUM
                    nc.scalar.activation(
                        out=pt[:],
                        in_=st[:],
                        func=mybir.ActivationFunctionType.Exp,
                        scale=float(1.0 / np.sqrt(D)),
                    )
                    # fillers AFTER scores/exp: the exp stream leads, the
                    # attn@v for the previous k-tile and any pulled
                    # projection work keep the PE busy under it.  (att0
                    # instead pulls BEFORE scores: its first scores wait
                    # on the RoPE chain, and the 4-deep engine wait queues
                    # would otherwise park the whole stream behind them.)
                    if not pulls_first and pulls is not None:
                        pulls(kt)
                    if prev_pt is not None:
                        pv(*prev_pt, stop=False)
                    prev_pt = (kt, pt)
                pv(*prev_pt, stop=True)
                if not fast_norm:
                    pending.append([norm_s1(ot, W), h, q0, W, 1])
                    return
                # fast close for the final attend: serial reciprocal on
                # DVE + a K=1 ones-matmul broadcast on the PE replace the
                # 4-hop DRAM bounce (dummy matmuls emitted by the caller
                # keep the PE p-state up during the ~4.5us DVE chain).
                den_sb = nrm_pool.tile([1, W], F32, tag="den", name="den_sb",
                                       padded_shape=[1, NQ])
                nc.vector.tensor_copy(out=den_sb[:], in_=ot[D : D + 1, :])
                rec16 = nrm_pool.tile([1, W], F16, tag="rec16", name="rec16",
                                      padded_shape=[1, NQ])
                nc.vector.reciprocal(out=rec16[:], in_=den_sb[:])
                ot_sb = nrm_pool.tile(
                    [D, W], F16, tag="otsb", name="ot_sb", bufs=3,
                    padded_shape=[D, NQ],
                )
                nc.vector.tensor_copy(out=ot_sb[:], in_=ot[0:D, :])
                rb_ps = psum_pj.tile([D, W], F32, tag="pj", name="rb_ps")
                mm(rb_ps[:], ones16[:], rec16[:], start=True, stop=True)
                nc.vector.tensor_mul(
                    out=otn[m][r0 : r0 + D, q0 : q0 + W],
                    in0=ot_sb[:],
                    in1=rb_ps[:],
                )

            # ---- emission schedule --------------------------------------
            import itertools

            gk0 = project_qk_gen("k", kTr, 0)
            gq0 = project_qk_gen(
                "q", qTr, 0, pools=[psum_st, psum_st, psum_pj, psum_pj]
            )
            gv = project_v_gen()
            gk1 = project_qk_gen("k", kTr, 1, rot_pool_engine=True)
            gq1 = project_qk_gen("q", qTr, 1, rot_pool_engine=True)
            go0 = project_out_gen(0)
            go1 = project_out_gen(1, scalar_evict_from=8)

            def take(g, n):
                for _ in range(n):
                    next(g, None)

            # preamble: only what attend(0,0)'s first scores need — k
            # block 0 and q cols 0-1023 of the first head pair.  v and the
            # remaining k/q blocks drip into attend(0,0) with in-order
            # deadlines (scores(kt) needs k block kt//4 roped, attn@v(kt)
            # needs v tile kt evicted).
            # k blocks 0-1 and q blocks 0-1 only read x columns 0-1023
            # (the first x half); everything else drips into attend(0,0).
            take(gk0, 16)
            take(gq0, 16)
            # deferred preloads: behind the preamble RoPE rots in ring
            # order, ahead of everything the attends pull in.
            for c in range(IC_CH):
                nc.scalar.dma_start(
                    out=w_sbs["v"][:, c, :], in_=wv_t[c * P : (c + 1) * P, :]
                )
            for c in (0, 2, 4, 6):
                nc.sync.dma_start(**xh(c, 1))
            for c in (1, 3, 5, 7):
                nc.scalar.dma_start(**xh(c, 1))
            for t in (2, 3):
                cs = slice(t * TB, (t + 1) * TB)
                nc.scalar.dma_start(out=cos_sb[t][:], in_=cos2[:, cs])
                nc.scalar.dma_start(out=sin_sb[t][:], in_=sin2[:, cs])
            chain_a = itertools.chain(gk0, gq0)

            def pulls_00(kt):
                take(gv, 8 if kt < 2 else 4)
                if kt < 8:
                    take(chain_a, 4)

            def drip(gen, kts, n=1):
                def pulls(kt):
                    if kt in kts:
                        take(gen, n)

                return pulls

            # Filler placement keeps the PE dense in EVERY attend (an idle
            # PE falls out of its 2.4GHz p-state, and at 1.2GHz the
            # attend's own matmuls no longer fit under the exp stream).
            # (3,0) runs before (2,1) so the first output half (ready
            # after (3,0)'s norm) fills the last attends; the final attend
            # is split into two 512-col halves so even most of the second
            # output half overlaps attention.
            allkt = tuple(range(NKT))
            attend(0, 0, NQ, pulls_00)
            attend(0, NQ, NQ, drip(gk1, allkt))
            nc.sync.dma_start(
                out=wo_sb[:], in_=wo_t.rearrange("(c p) o -> p c o", p=P)
            )
            attend(1, 0, NQ, drip(gk1, allkt))
            attend(1, NQ, NQ, drip(gq1, allkt))
            attend(2, 0, NQ, drip(gq1, allkt))
            attend(3, 0, NQ)
            attend(2, NQ, NQ, drip(go0, (8, 9, 10, 11, 12, 13, 14, 15), n=2))
            attend(3, NQ, TB, drip(go0, allkt))
            take(go0, 32)  # drain any remainder
            attend(3, NQ + TB, TB,
                   drip(go1, (6, 7, 8, 9, 10, 11, 12, 13), n=2),
                   fast_norm=True)
            # tail: dummy matmuls keep the PE p-state up while the fast
            # normalization's DVE chain completes, then the last quarter
            # of the output projection (scalar-engine evicts).
            for i in range(10):
                dps = psum_st.tile([P, TB], F32, tag="st", name="dps",
                                   padded_shape=[P, NQ])
                mm(
                    dps[:],
                    kTr[i % 2][:, 0:P],
                    qTr[i % 2][:, 0:TB],
                    start=True,
                    stop=True,
                )
            take(go1, 32)
    _split_waits(nc)
    return nc


def _rope_tables():
    inv_freq = 1.0 / (10000.0 ** (np.arange(0, D, 2, dtype=np.float32) / D))
    t = np.arange(T, dtype=np.float32)
    freqs = t[:, None] * inv_freq[None, :]  # [T, 32]
    emb = np.concatenate((freqs, freqs), axis=-1)  # [T, 64]
    cos = np.cos(emb).T.astype(np.float32)  # [64, T]
    sin = np.sin(emb).T.astype(np.float32)
    sign = np.where(np.arange(D) < D // 2, -1.0, 1.0).astype(np.float32)
    sin_signed = sin * sign[:, None]
    cos2 = np.ascontiguousarray(np.concatenate([cos, cos], axis=0))  # [128,T]
    sin2 = np.ascontiguousarray(np.concatenate([sin_signed, sin_signed], 0))
    return cos2, sin2


def make_in_maps(x, wq, wk, wv, wo):
    f16 = np.float16
    cos2, sin2 = _rope_tables()
    in_maps = []
    for core in range(N_CORES):
        b, g = divmod(core, N_CORES // B)
        hs = slice(g * HD, (g + 1) * HD)
        wq_t = np.ascontiguousarray(wq[hs].T).astype(f16)
        wk_t = np.ascontiguousarray(wk[hs].T).astype(f16)
        in_maps.append(
            {
                "xT": np.ascontiguousarray(x[b].T).astype(f16),
                "wq_t": wq_t,
                "wk_t": wk_t,
                "wv_t": np.ascontiguousarray(wv[hs].T).astype(f16),
                "wo_t": np.ascontiguousarray(wo[:, hs].T).astype(f16),
                "cos2": cos2.astype(f16),
                "sin2": sin2.astype(f16),
            }
        )
    return in_maps


def gather_output(results):
    y = np.zeros((B, T, HIDDEN), dtype=np.float32)
    for core, res in enumerate(results):
        b = core // (N_CORES // B)
        y[b] += res["yT"].T.astype(np.float32)
    return y


_NC = None


def kernel(x, wq, wk, wv, wo):
    global _NC
    import time

    from concourse.bass_utils import run_bass_kernel_spmd

    if _NC is None:
        _NC = build_kernel()
    in_maps = make_in_maps(
        np.asarray(x), np.asarray(wq), np.asarray(wk), np.asarray(wv), np.asarray(wo)
    )
    try:
        res = run_bass_kernel_spmd(_NC, in_maps, core_ids=list(range(N_CORES)))
    except Exception:
        # transient device wedge (e.g. NRT_EXEC_UNIT_UNRECOVERABLE from a
        # prior run) -- retry once
        time.sleep(2.0)
        res = run_bass_kernel_spmd(_NC, in_maps, core_ids=list(range(N_CORES)))
    return gather_output(res.results)
